# revision 1
# baseline (speedup 1.0000x reference)
"""DCT-II enhancement kernel for Trainium2 (8 NeuronCores, data parallel).

Computes out[b, n, k] = sum_d x[b, n, d] * C[k, d] where C is the 256x256
orthonormal DCT-II basis — i.e. a [B*N, 256] @ [256, 256]^T GEMM.

Sharding: pure data parallel over the flattened token dim (B*N = 131072),
16384 tokens per core. The DCT basis (transposed, [d, k]) and a 128x128
identity (for PE-transpose) are replicated to every core.

Per-core dataflow, per 512-token super-tile:
  1. DMA x tile [128p(tok), 4t, 256d] from HBM (natural layout, contiguous).
  2. PE-transpose (fp32r) the 8 [128, 128] blocks -> xT in PSUM [d, tok].
  3. Copy PSUM -> SBUF (DVE).
  4. fp32r matmuls: out[tok=128, k=256] += xT_chunk.T @ CT_chunk for the
     two 128-deep d-chunks (moving free dim 256 -> full-rate fp32r).
  5. Copy PSUM -> SBUF (DVE/ACT), DMA out to HBM in natural layout.
"""

from contextlib import ExitStack

import numpy as np

import concourse.bass as bass
import concourse.tile as tile
from concourse import bacc, mybir
from concourse.bass_utils import run_bass_kernel_spmd

P = 128
D = 256
N_CORES = 8
B, N = 32, 4096
TOK_PER_CORE = (B * N) // N_CORES  # 16384

F32 = mybir.dt.float32
F32R = mybir.dt.float32r


def dct_matrix() -> np.ndarray:
    """C[k, d] — DCT-II with ortho normalization, fp64 math cast to fp32."""
    n = D
    k = np.arange(n)[:, None].astype(np.float64)
    m = np.arange(n)[None, :].astype(np.float64)
    Cm = np.cos(np.pi * (2.0 * m + 1.0) * k / (2.0 * n))
    scale = np.full((n, 1), np.sqrt(2.0 / n))
    scale[0, 0] = np.sqrt(1.0 / n)
    return (Cm * scale).astype(np.float32)


def build_program(tok: int = TOK_PER_CORE, super_tok: int = 512,
                  num_devices: int = N_CORES) -> bass.Bass:
    """Emit the per-core Bass/Tile program. All cores run the same NEFF.

    Layout: token = i*super_tok + p*tb + s  (tb tokens per partition, so
    each partition's DMA run is tb*D*4 bytes contiguous — 4 KB at tb=4,
    512 KB per dma_start, alternating between the two HWDGE rings).

    Pipeline (3 stages, 2-iteration decoupling at every hop):
      A(i): DMA in                           (lead 3)
      B(i): 8 PE transposes -> 2 PSUM banks -> 2 SBUF copies
      C(i): 8 fp32r matmuls -> 2 PSUM banks (2 accum groups per bank)
            -> 2 SBUF copies -> DMA out
    PSUM: xt pool 4 x [128,512] banks (2/iter), out pool 4 x [128,512]
    banks (2/iter) — both 2 iterations deep. PE sees one 8-transpose
    burst then one 8-matmul burst per slot (2 mode switches).
    Copies alternate DVE/ACT by iteration parity to balance their
    measured PSUM-read rates (~1.34 vs ~2.6 ns/elem).
    """
    assert tok % super_tok == 0 and super_tok % (2 * P) == 0
    nit = tok // super_tok   # super-tile iterations
    tb = super_tok // P      # tokens per partition per super-tile
    dc = D // P              # d-chunks (contraction over 2x128)

    nc = bacc.Bacc(
        "TRN2", target_bir_lowering=False, debug=False, num_devices=num_devices
    )
    x_d = nc.dram_tensor("x", [tok, D], F32, kind="ExternalInput").ap()
    ct_d = nc.dram_tensor("ct", [D, D], F32, kind="ExternalInput").ap()
    id_d = nc.dram_tensor("ident", [P, P], F32, kind="ExternalInput").ap()
    out_d = nc.dram_tensor("out", [tok, D], F32, kind="ExternalOutput").ap()

    with ExitStack() as ctx:
        tc = ctx.enter_context(tile.TileContext(nc))
        consts = ctx.enter_context(tc.tile_pool(name="consts", bufs=1))
        xin_pool = ctx.enter_context(tc.tile_pool(name="xin", bufs=8))
        xt_sb_pool = ctx.enter_context(tc.tile_pool(name="xt_sb", bufs=4))
        out_sb_pool = ctx.enter_context(tc.tile_pool(name="out_sb", bufs=6))
        xt_ps_pool = ctx.enter_context(
            tc.tile_pool(name="xt_ps", bufs=3, space="PSUM")
        )
        out_ps_pool = ctx.enter_context(
            tc.tile_pool(name="out_ps", bufs=5, space="PSUM")
        )

        # Replicated constants: CT as [p, c, k] (d = c*128 + p), identity.
        # ident first on the sync ring (needed by the first transpose);
        # ct on the scalar ring (first needed ~10us in, keeps sync free).
        ident = consts.tile([P, P], F32R)
        nc.sync.dma_start(ident[:], id_d.bitcast(F32R))
        ct_sb = consts.tile([P, dc, D], F32R)

        def load_ct():
            nc.scalar.dma_start(
                ct_sb[:], ct_d.rearrange("(c p) k -> p c k", p=P).bitcast(F32R)
            )

        # token = i*super_tok + p*tb + s -> per-partition contiguous tb*D run
        x_t = x_d.rearrange("(i p s) d -> i p s d", p=P, s=tb)
        o_t = out_d.rearrange("(i p s) k -> i p s k", p=P, s=tb)

        rings = [nc.sync, nc.scalar]

        xins = {}
        xts = {}

        def stage_a(i):
            if not (0 <= i < nit):
                return
            if i == 0:
                # Pipeline fill: land iteration 0 as 4 per-chunk tiles with
                # precise deps so the first transpose starts ~4us earlier.
                chunks = []
                for s in range(tb):
                    xc = xin_pool.tile([P, 1, D], F32R, name=f"xin0_{s}")
                    nc.sync.dma_start(
                        xc[:], x_t[0, :, s:s + 1, :].bitcast(F32R)
                    )
                    chunks.append(xc)
                xins[i] = chunks
                return
            xin = xin_pool.tile([P, tb, D], F32R)
            # Split the input stream across two issue paths: HWDGE (sync)
            # and SWDGE (gpsimd, otherwise idle) so each SDMA engine has
            # two read queues to interleave at packet granularity.
            eng = nc.gpsimd if i % 2 == 1 else nc.sync
            eng.dma_start(xin[:], x_t[i].bitcast(F32R))
            xins[i] = xin

        def copy(engine, dst, src):
            if engine == "act":
                nc.scalar.copy(dst, src)
            else:
                nc.vector.tensor_copy(dst, src)

        def stage_b(i):
            """Transposes (one 8-burst) + xT PSUM->SBUF copies."""
            if not (0 <= i < nit):
                return
            xin = xins.pop(i)

            def xin_slice(s, c):
                if isinstance(xin, list):
                    return xin[s][:, 0, c * P:(c + 1) * P]
                return xin[:, s, c * P:(c + 1) * P]

            xt_sb = xt_sb_pool.tile([P, dc, super_tok], F32R)
            xts[i] = xt_sb
            xt_pss = []
            for c in range(dc):
                xt_ps = xt_ps_pool.tile([P, super_tok], F32R)
                xt_pss.append(xt_ps)
                for s in range(tb):
                    nc.tensor.transpose(
                        xt_ps[:, s * P:(s + 1) * P],
                        xin_slice(s, c),
                        ident[:],
                    )
            # xT copies gate the matmuls -> always on the fast DVE.
            copy("dve", xt_sb[:, 0, :], xt_pss[0][:])
            copy("dve", xt_sb[:, 1, :], xt_pss[1][:])

        def stage_c(i):
            """Matmuls (one 8-burst into 2 banks) + out copies + DMA out."""
            if not (0 <= i < nit):
                return
            xt_sb = xts.pop(i)
            out_sb = out_sb_pool.tile([P, tb, D], F32)
            out_pss = []
            for sp in range(tb // 2):
                out_ps = out_ps_pool.tile([P, 2 * D], F32)
                out_pss.append(out_ps)
                for s_in in range(2):
                    s = 2 * sp + s_in
                    for c in range(dc):
                        nc.tensor.matmul(
                            out_ps[:, s_in * D:(s_in + 1) * D],
                            xt_sb[:, c, s * P:(s + 1) * P],
                            ct_sb[:, c, :],
                            start=(c == 0),
                            stop=(c == dc - 1),
                        )
            # Out copies have ~2 iterations of slack (PSUM depth + out_sb
            # bufs): balance DVE/ACT by alternating the first copy's engine.
            eng0 = "act" if i % 2 == 0 else "dve"
            copy(eng0, out_sb[:, 0:2, :], out_pss[0][:])
            if i >= nit - 2:
                # Drain the tail sooner: ship each half as soon as copied.
                nc.scalar.dma_start(o_t[i, :, 0:2, :], out_sb[:, 0:2, :])
                copy("act", out_sb[:, 2:4, :], out_pss[1][:])
                nc.scalar.dma_start(o_t[i, :, 2:4, :], out_sb[:, 2:4, :])
            else:
                copy("act", out_sb[:, 2:4, :], out_pss[1][:])
                nc.scalar.dma_start(o_t[i], out_sb[:])

        stage_a(0)
        load_ct()
        stage_a(1)
        stage_a(2)
        for i in range(nit + 1):
            stage_a(i + 3)
            stage_b(i)
            stage_c(i - 1)

    nc.compile()
    return nc


_PROGRAM_CACHE: dict = {}


def _get_program() -> bass.Bass:
    if "nc" not in _PROGRAM_CACHE:
        _PROGRAM_CACHE["nc"] = build_program()
    return _PROGRAM_CACHE["nc"]


def make_in_maps(x_flat: np.ndarray) -> list[dict]:
    ct = np.ascontiguousarray(dct_matrix().T)  # [d, k]
    ident = np.eye(P, dtype=np.float32)
    shards = x_flat.reshape(N_CORES, TOK_PER_CORE, D)
    return [
        {"x": np.ascontiguousarray(shards[i]), "ct": ct, "ident": ident}
        for i in range(N_CORES)
    ]


def kernel(x: np.ndarray) -> np.ndarray:
    x = np.ascontiguousarray(np.asarray(x, dtype=np.float32))
    b, n, d = x.shape
    assert (b, n, d) == (B, N, D), f"unexpected shape {x.shape}"
    nc = _get_program()
    in_maps = make_in_maps(x.reshape(b * n, d))
    res = run_bass_kernel_spmd(nc, in_maps, core_ids=list(range(N_CORES)))
    out = np.concatenate([r["out"] for r in res.results], axis=0)
    return out.reshape(b, n, d)



# revision 3
# speedup vs baseline: 1.4048x; 1.4048x over previous
"""DCT-II enhancement kernel for Trainium2 (8 NeuronCores, data parallel).

Computes out[b, n, k] = sum_d x[b, n, d] * C[k, d] where C is the 256x256
orthonormal DCT-II basis — i.e. a [B*N, 256] @ [256, 256]^T GEMM.

Sharding: pure data parallel over the flattened token dim (B*N = 131072),
16384 tokens per core. The DCT basis (transposed, [d, k]) is replicated.

Precision: the correctness gate is rel_err < 2e-2; an orthonormal basis
preserves norms, so bf16 I/O contributes only ~3e-3 relative error while
halving HBM traffic (the fp32 version of this kernel sat exactly on the
fp32 DMA roofline at ~103us; the bf16 floor is ~50us/core at ~330 GB/s).

Layout: the host pre-transposes each core's shard to xT [d=256, tok=16384]
bf16, with tokens permuted so that the device's natural tiling (psum block
s of iteration i holds tokens i*512 + p*4 + s at partition p) writes the
output in natural row-major token order. The device therefore runs a pure
matmul pipeline — no PE transposes, no xT PSUM->SBUF staging:

  per 512-token iteration:
    1. DMA in xT tile [128p(d), 2c, tok] bf16 (2KB/partition runs, fetched
       in 1024-token granules, alternating HWDGE(sync)/SWDGE(gpsimd)).
    2. 8 matmuls into 2 PSUM banks: out[tok=128, k=256] += xTchunk.T @ CT
       (lhsT = xT slice, 128d x 128tok; rhs = CT chunk, 128d x 256k).
    3. 2 PSUM->SBUF copies with fp32->bf16 cast (DVE + ACT, one each).
    4. DMA out [128p, 4, 256] bf16 (2KB contiguous per partition) on the
       scalar ring.
"""

from contextlib import ExitStack

import ml_dtypes
import numpy as np

import concourse.bass as bass
import concourse.tile as tile
from concourse import bacc, mybir
from concourse.bass_utils import run_bass_kernel_spmd

P = 128
D = 256
N_CORES = 8
B, N = 32, 4096
TOK_PER_CORE = (B * N) // N_CORES  # 16384

F32 = mybir.dt.float32
BF16 = mybir.dt.bfloat16
NP_BF16 = ml_dtypes.bfloat16


def dct_matrix() -> np.ndarray:
    """C[k, d] — DCT-II with ortho normalization, fp64 math cast to fp32."""
    n = D
    k = np.arange(n)[:, None].astype(np.float64)
    m = np.arange(n)[None, :].astype(np.float64)
    Cm = np.cos(np.pi * (2.0 * m + 1.0) * k / (2.0 * n))
    scale = np.full((n, 1), np.sqrt(2.0 / n))
    scale[0, 0] = np.sqrt(1.0 / n)
    return (Cm * scale).astype(np.float32)


def build_program(tok: int = TOK_PER_CORE, super_tok: int = 512,
                  num_devices: int = N_CORES) -> bass.Bass:
    """Emit the per-core Bass/Tile program. All cores run the same NEFF."""
    assert tok % super_tok == 0 and super_tok % (2 * P) == 0
    nit = tok // super_tok   # 512-token iterations (32)
    tb = super_tok // P      # tokens per output partition per iter (4)
    dc = D // P              # contraction chunks (2 x 128)
    gr = 2 * super_tok       # input DMA granule: 1024 tokens, 2KB runs
    ngr = tok // gr

    nc = bacc.Bacc(
        "TRN2", target_bir_lowering=False, debug=False, num_devices=num_devices
    )
    xt_d = nc.dram_tensor("xt", [D, tok], BF16, kind="ExternalInput").ap()
    ct_d = nc.dram_tensor("ct", [D, D], BF16, kind="ExternalInput").ap()
    out_d = nc.dram_tensor("out", [tok, D], BF16, kind="ExternalOutput").ap()

    with ExitStack() as ctx:
        tc = ctx.enter_context(tile.TileContext(nc))
        consts = ctx.enter_context(tc.tile_pool(name="consts", bufs=1))
        xin_pool = ctx.enter_context(tc.tile_pool(name="xin", bufs=8))
        out_sb_pool = ctx.enter_context(tc.tile_pool(name="out_sb", bufs=6))
        out_ps_pool = ctx.enter_context(
            tc.tile_pool(name="out_ps", bufs=8, space="PSUM")
        )

        # CT as [p, c, k] (d = c*128 + p), first on the scalar ring — the
        # very first matmul needs it.
        ct_sb = consts.tile([P, dc, D], BF16)
        nc.scalar.dma_start(ct_sb[:], ct_d.rearrange("(c p) k -> p c k", p=P))

        # xT column j of iteration i holds token i*512 + p*4 + s where
        # j = s*128 + p (host-side permutation) -> psum block s lands
        # tokens contiguously per partition for the out DMA.
        x_fill = xt_d.rearrange("(c p) (i t) -> i p c t", p=P, t=super_tok)
        x_gr = xt_d.rearrange("(c p) (g t) -> g p c t", p=P, t=gr)
        o_t = out_d.rearrange("(i p s) k -> i p s k", p=P, s=tb)

        xins = {}

        def stage_a_fill(i):
            """Iterations 0-1 land as standalone 512-token tiles so the
            first matmuls start as early as possible."""
            xc = xin_pool.tile([P, dc, super_tok], BF16, name=f"xfill{i}")
            nc.sync.dma_start(xc[:], x_fill[i])
            xins[i] = (xc, 0)

        def stage_a(g):
            """Granule g covers iterations 2g, 2g+1 (g >= 1)."""
            if not (1 <= g < ngr):
                return
            xg = xin_pool.tile([P, dc, gr], BF16)
            eng = nc.gpsimd if g % 2 == 1 else nc.sync
            eng.dma_start(xg[:], x_gr[g])
            xins[2 * g] = (xg, 0)
            xins[2 * g + 1] = (xg, super_tok)

        def stage_b(i):
            """8 matmuls -> 2 PSUM banks (2 accumulation groups each)."""
            if not (0 <= i < nit):
                return
            xin, off = xins.pop(i)
            pss = []
            for sp in range(tb // 2):
                ps = out_ps_pool.tile([P, 2 * D], F32)
                pss.append(ps)
                for s_in in range(2):
                    s = 2 * sp + s_in
                    w0 = off + s * P
                    for c in range(dc):
                        nc.tensor.matmul(
                            ps[:, s_in * D:(s_in + 1) * D],
                            xin[:, c, w0:w0 + P],
                            ct_sb[:, c, :],
                            start=(c == 0),
                            stop=(c == dc - 1),
                        )
            pss_by_iter[i] = pss

        pss_by_iter = {}

        def stage_c(i):
            """PSUM->SBUF bf16 copies (DVE + ACT) + out DMA (scalar ring)."""
            if not (0 <= i < nit):
                return
            pss = pss_by_iter.pop(i)
            out_sb = out_sb_pool.tile([P, tb, D], BF16)
            nc.vector.tensor_copy(out_sb[:, 0:2, :], pss[0][:])
            nc.scalar.copy(out_sb[:, 2:4, :], pss[1][:])
            nc.scalar.dma_start(o_t[i], out_sb[:])

        stage_a_fill(0)
        stage_a_fill(1)
        stage_a(1)
        stage_a(2)
        for i in range(nit + 1):
            if i % 2 == 0:
                stage_a(i // 2 + 3)
            stage_b(i)
            stage_c(i - 1)

    nc.compile()
    return nc


_PROGRAM_CACHE: dict = {}


def _get_program() -> bass.Bass:
    if "nc" not in _PROGRAM_CACHE:
        _PROGRAM_CACHE["nc"] = build_program()
    return _PROGRAM_CACHE["nc"]


def make_in_maps(x_flat: np.ndarray) -> list[dict]:
    """x_flat: [B*N, D] float32. Cast to bf16 and pre-transpose each shard
    to xT [d, j] where column j = i*512 + s*128 + p holds token
    i*512 + p*4 + s (matches the device's psum-block tiling)."""
    ct = np.ascontiguousarray(dct_matrix().T).astype(NP_BF16)  # [d, k]
    xb = x_flat.astype(NP_BF16)
    nit = TOK_PER_CORE // 512
    # [core, i, p, s, d] -> [core, d, i, s, p]
    xr = xb.reshape(N_CORES, nit, P, 4, D).transpose(0, 4, 1, 3, 2)
    xt = np.ascontiguousarray(xr).reshape(N_CORES, D, TOK_PER_CORE)
    return [{"xt": xt[i], "ct": ct} for i in range(N_CORES)]


def kernel(x: np.ndarray) -> np.ndarray:
    x = np.ascontiguousarray(np.asarray(x, dtype=np.float32))
    b, n, d = x.shape
    assert (b, n, d) == (B, N, D), f"unexpected shape {x.shape}"
    nc = _get_program()
    in_maps = make_in_maps(x.reshape(b * n, d))
    res = run_bass_kernel_spmd(nc, in_maps, core_ids=list(range(N_CORES)))
    out = np.concatenate(
        [np.asarray(r["out"]).astype(np.float32) for r in res.results], axis=0
    )
    return out.reshape(b, n, d)


# revision 7
# speedup vs baseline: 1.6032x; 1.1412x over previous
"""DCT-II enhancement kernel for Trainium2 (8 NeuronCores, data parallel).

Computes out[b, n, k] = sum_d x[b, n, d] * C[k, d] where C is the 256x256
orthonormal DCT-II basis — i.e. a [B*N, 256] @ [256, 256]^T GEMM.

Sharding: pure data parallel over the flattened token dim (B*N = 131072),
16384 tokens per core. The DCT basis (transposed, [d, k]) is replicated.

Precision: the correctness gate is rel_err < 2e-2; the orthonormal basis
preserves norms, so bf16 I/O contributes only ~3e-3 relative error while
halving HBM traffic (the fp32 version sat on the fp32 DMA roofline at
~103us; bf16 floor is ~51us/core at ~330 GB/s).

Key bottleneck learned from traces: a single DMA ring sustains only
~110-145 GB/s, and only three rings exist (HWDGE on sync/scalar, SWDGE
on gpsimd). Traffic is balanced ~5.5MB/ring: input on gpsimd + sync,
output on scalar with sync absorbing the late iterations (when its input
work is done), 4KB descriptors everywhere (1024-token iterations).

Layout: the host pre-transposes each core's shard to xT [d=256, 16384]
bf16, tokens permuted so the device's natural tiling (psum block s of
iteration I holds token I*1024 + p*8 + s at partition p) writes the
output in natural row-major order. Device = pure matmul pipeline, no PE
transposes:

  per 1024-token iteration:
    1. DMA in xT [128p(d), 2c, tok] bf16 (4KB runs, 2048-token granules,
       alternating sync/gpsimd rings).
    2. 16 matmuls into 4 PSUM banks: out[tok=128, k=256] += xTc.T @ CTc
       (lhsT = xT slice 128d x 128tok, rhs = CT chunk 128d x 256k).
    3. 4 PSUM->SBUF copies with fp32->bf16 cast (2 on DVE, 2 on ACT).
    4. DMA out [128p, 8, 256] bf16 (4KB contiguous per partition),
       alternating scalar/vector rings; last iterations ship each half
       as soon as its copies land to shorten the tail drain.
"""

from contextlib import ExitStack

import ml_dtypes
import numpy as np

import concourse.bass as bass
import concourse.tile as tile
from concourse import bacc, mybir
from concourse.bass_utils import run_bass_kernel_spmd

P = 128
D = 256
N_CORES = 8
B, N = 32, 4096
TOK_PER_CORE = (B * N) // N_CORES  # 16384

F32 = mybir.dt.float32
BF16 = mybir.dt.bfloat16
NP_BF16 = ml_dtypes.bfloat16


def dct_matrix() -> np.ndarray:
    """C[k, d] — DCT-II with ortho normalization, fp64 math cast to fp32."""
    n = D
    k = np.arange(n)[:, None].astype(np.float64)
    m = np.arange(n)[None, :].astype(np.float64)
    Cm = np.cos(np.pi * (2.0 * m + 1.0) * k / (2.0 * n))
    scale = np.full((n, 1), np.sqrt(2.0 / n))
    scale[0, 0] = np.sqrt(1.0 / n)
    return (Cm * scale).astype(np.float32)


def build_program(tok: int = TOK_PER_CORE, super_tok: int = 1024,
                  num_devices: int = N_CORES) -> bass.Bass:
    """Emit the per-core Bass/Tile program. All cores run the same NEFF."""
    assert tok % super_tok == 0 and super_tok % (2 * P) == 0
    nit = tok // super_tok       # 1024-token iterations (16)
    tb = super_tok // P          # tokens per output partition per iter (8)
    dc = D // P                  # contraction chunks (2 x 128)
    gr = 2 * super_tok           # input DMA granule: 2048 tokens, 4KB runs
    ngr = tok // gr

    nc = bacc.Bacc(
        "TRN2", target_bir_lowering=False, debug=False, num_devices=num_devices
    )
    xt_d = nc.dram_tensor("xt", [D, tok], BF16, kind="ExternalInput").ap()
    ct_d = nc.dram_tensor("ct", [D, D], BF16, kind="ExternalInput").ap()
    out_d = nc.dram_tensor("out", [tok, D], BF16, kind="ExternalOutput").ap()

    with ExitStack() as ctx:
        tc = ctx.enter_context(tile.TileContext(nc))
        consts = ctx.enter_context(tc.tile_pool(name="consts", bufs=1))
        xin_pool = ctx.enter_context(tc.tile_pool(name="xin", bufs=6))
        out_sb_pool = ctx.enter_context(tc.tile_pool(name="out_sb", bufs=4))
        out_ps_pool = ctx.enter_context(
            tc.tile_pool(name="out_ps", bufs=8, space="PSUM")
        )

        # CT as [p, c, k] (d = c*128 + p), first on the scalar ring — the
        # very first matmul needs it.
        ct_sb = consts.tile([P, dc, D], BF16)
        nc.scalar.dma_start(ct_sb[:], ct_d.rearrange("(c p) k -> p c k", p=P))

        # xT column j of iteration I holds token I*1024 + p*8 + s where
        # j = s*128 + p (host-side permutation) -> psum block s lands
        # tokens contiguously per partition for 4KB-run out DMAs.
        x_half = xt_d.rearrange("(c p) (h t) -> h p c t", p=P, t=super_tok // 2)
        x_fill = xt_d.rearrange("(c p) (i t) -> i p c t", p=P, t=super_tok)
        x_gr = xt_d.rearrange("(c p) (g t) -> g p c t", p=P, t=gr)
        o_t = out_d.rearrange("(i p s) k -> i p s k", p=P, s=tb)

        xins = {}

        def stage_a_fill0():
            """Iteration 0 lands as two 512-token half tiles so the first
            matmuls start after only 256KB of input."""
            ha = xin_pool.tile([P, dc, super_tok // 2], BF16, name="xf0a")
            hb = xin_pool.tile([P, dc, super_tok // 2], BF16, name="xf0b")
            nc.sync.dma_start(ha[:], x_half[0])
            nc.sync.dma_start(hb[:], x_half[1])
            xins[0] = ("pair", ha, hb)

        def stage_a_fill(i, eng):
            xc = xin_pool.tile([P, dc, super_tok], BF16, name=f"xfill{i}")
            eng.dma_start(xc[:], x_fill[i])
            xins[i] = ("one", xc, 0)

        # Granule ring schedule: sync takes g2 and g5 (its fill work ends
        # early), gpsimd the rest; sync then absorbs late out DMAs.
        GR_SYNC = {2, 5}

        def stage_a(g):
            """Granule g covers iterations 2g, 2g+1 (g >= 1)."""
            if not (1 <= g < ngr):
                return
            xg = xin_pool.tile([P, dc, gr], BF16)
            eng = nc.sync if g in GR_SYNC else nc.gpsimd
            eng.dma_start(xg[:], x_gr[g])
            xins[2 * g] = ("one", xg, 0)
            xins[2 * g + 1] = ("one", xg, super_tok)

        pss_by_iter = {}

        def stage_b(i):
            """16 matmuls -> 4 PSUM banks (2 accumulation groups each)."""
            if not (0 <= i < nit):
                return
            ent = xins.pop(i)
            if ent[0] == "pair":
                _, ha, hb = ent
                half = tb // 2

                def wslice(c, s):
                    t = ha if s < half else hb
                    o = (s % half) * P
                    return t[:, c, o:o + P]
            else:
                _, xg, off = ent

                def wslice(c, s):
                    o = off + s * P
                    return xg[:, c, o:o + P]

            pss = []
            for sp in range(tb // 2):
                ps = out_ps_pool.tile([P, 2 * D], F32)
                pss.append(ps)
                for s_in in range(2):
                    s = 2 * sp + s_in
                    for c in range(dc):
                        nc.tensor.matmul(
                            ps[:, s_in * D:(s_in + 1) * D],
                            wslice(c, s),
                            ct_sb[:, c, :],
                            start=(c == 0),
                            stop=(c == dc - 1),
                        )
            pss_by_iter[i] = pss

        # Out-DMA ring per iteration: scalar by default; sync takes the
        # late iterations once its input granules are through.
        OUT_SYNC = {9, 11, 13}

        def stage_c(i):
            """PSUM->SBUF bf16 copies (2 DVE + 2 ACT) + out DMA."""
            if not (0 <= i < nit):
                return
            pss = pss_by_iter.pop(i)
            out_sb = out_sb_pool.tile([P, tb, D], BF16)
            ring = nc.sync if i in OUT_SYNC else nc.scalar
            nc.vector.tensor_copy(out_sb[:, 0:2, :], pss[0][:])
            nc.scalar.copy(out_sb[:, 2:4, :], pss[1][:])
            if i >= nit - 2:
                # Tail drain: ship each half as soon as its copies land,
                # on different rings.
                ringa = nc.scalar if i % 2 == 0 else nc.sync
                ringb = nc.sync if i % 2 == 0 else nc.scalar
                ringa.dma_start(o_t[i, :, 0:4, :], out_sb[:, 0:4, :])
                nc.vector.tensor_copy(out_sb[:, 4:6, :], pss[2][:])
                nc.scalar.copy(out_sb[:, 6:8, :], pss[3][:])
                ringb.dma_start(o_t[i, :, 4:8, :], out_sb[:, 4:8, :])
            else:
                nc.vector.tensor_copy(out_sb[:, 4:6, :], pss[2][:])
                nc.scalar.copy(out_sb[:, 6:8, :], pss[3][:])
                ring.dma_start(o_t[i], out_sb[:])

        stage_a_fill0()
        stage_a_fill(1, nc.gpsimd)
        stage_a(1)
        for i in range(nit + 1):
            if i % 2 == 0:
                stage_a(i // 2 + 2)
            stage_b(i)
            stage_c(i - 1)

    nc.compile()
    return nc


_PROGRAM_CACHE: dict = {}


def _get_program() -> bass.Bass:
    if "nc" not in _PROGRAM_CACHE:
        _PROGRAM_CACHE["nc"] = build_program()
    return _PROGRAM_CACHE["nc"]


def make_in_maps(x_flat: np.ndarray) -> list[dict]:
    """x_flat: [B*N, D] float32. Cast to bf16 and pre-transpose each shard
    to xT [d, j] where column j = I*1024 + s*128 + p holds token
    I*1024 + p*8 + s (matches the device's psum-block tiling)."""
    ct = np.ascontiguousarray(dct_matrix().T).astype(NP_BF16)  # [d, k]
    xb = x_flat.astype(NP_BF16)
    nit = TOK_PER_CORE // 1024
    # [core, I, p, s, d] -> [core, d, I, s, p]
    xr = xb.reshape(N_CORES, nit, P, 8, D).transpose(0, 4, 1, 3, 2)
    xt = np.ascontiguousarray(xr).reshape(N_CORES, D, TOK_PER_CORE)
    return [{"xt": xt[i], "ct": ct} for i in range(N_CORES)]


def kernel(x: np.ndarray) -> np.ndarray:
    x = np.ascontiguousarray(np.asarray(x, dtype=np.float32))
    b, n, d = x.shape
    assert (b, n, d) == (B, N, D), f"unexpected shape {x.shape}"
    nc = _get_program()
    in_maps = make_in_maps(x.reshape(b * n, d))
    res = run_bass_kernel_spmd(nc, in_maps, core_ids=list(range(N_CORES)))
    out = np.concatenate(
        [np.asarray(r["out"]).astype(np.float32) for r in res.results], axis=0
    )
    return out.reshape(b, n, d)


# revision 13
# speedup vs baseline: 1.6645x; 1.0382x over previous
"""DCT-II enhancement kernel for Trainium2 (8 NeuronCores, data parallel).

Computes out[b, n, k] = sum_d x[b, n, d] * C[k, d] where C is the 256x256
orthonormal DCT-II basis — i.e. a [B*N, 256] @ [256, 256]^T GEMM.

Sharding: pure data parallel over the flattened token dim (B*N = 131072),
16384 tokens per core. The DCT basis (transposed, [d, k]) is replicated.

Precision: the correctness gate is rel_err < 2e-2; the orthonormal basis
preserves norms, so bf16 I/O contributes only ~3e-3 relative error while
halving HBM traffic (the fp32 version sat on the fp32 DMA roofline at
~103us; bf16 floor is ~51us/core at ~330 GB/s).

Key bottleneck learned from traces: a single DMA ring sustains only
~110-145 GB/s, and only three rings exist (HWDGE on sync/scalar, SWDGE
on gpsimd). Traffic is balanced ~5.5MB/ring: input on gpsimd + sync,
output on scalar with sync absorbing the late iterations (when its input
work is done), 4KB descriptors everywhere (1024-token iterations).

Layout: the host pre-transposes each core's shard to xT [d=256, 16384]
bf16, tokens permuted so the device's natural tiling (psum block s of
iteration I holds token I*1024 + p*8 + s at partition p) writes the
output in natural row-major order. Device = pure matmul pipeline, no PE
transposes:

  per 1024-token iteration:
    1. DMA in xT [128p(d), 2c, tok] bf16 (4KB runs, 2048-token granules,
       alternating sync/gpsimd rings).
    2. 16 matmuls into 4 PSUM banks: out[tok=128, k=256] += xTc.T @ CTc
       (lhsT = xT slice 128d x 128tok, rhs = CT chunk 128d x 256k).
    3. 4 PSUM->SBUF copies with fp32->bf16 cast (2 on DVE, 2 on ACT).
    4. DMA out [128p, 8, 256] bf16 (4KB contiguous per partition),
       alternating scalar/vector rings; last iterations ship each half
       as soon as its copies land to shorten the tail drain.
"""

from contextlib import ExitStack

import ml_dtypes
import numpy as np

import concourse.bass as bass
import concourse.tile as tile
from concourse import bacc, mybir
from concourse.bass_utils import run_bass_kernel_spmd

P = 128
D = 256
N_CORES = 8
B, N = 32, 4096
TOK_PER_CORE = (B * N) // N_CORES  # 16384

F32 = mybir.dt.float32
BF16 = mybir.dt.bfloat16
NP_BF16 = ml_dtypes.bfloat16


def dct_matrix() -> np.ndarray:
    """C[k, d] — DCT-II with ortho normalization, fp64 math cast to fp32."""
    n = D
    k = np.arange(n)[:, None].astype(np.float64)
    m = np.arange(n)[None, :].astype(np.float64)
    Cm = np.cos(np.pi * (2.0 * m + 1.0) * k / (2.0 * n))
    scale = np.full((n, 1), np.sqrt(2.0 / n))
    scale[0, 0] = np.sqrt(1.0 / n)
    return (Cm * scale).astype(np.float32)


def build_program(tok: int = TOK_PER_CORE, super_tok: int = 1024,
                  num_devices: int = N_CORES) -> bass.Bass:
    """Emit the per-core Bass/Tile program. All cores run the same NEFF."""
    assert tok % super_tok == 0 and super_tok % (2 * P) == 0
    nit = tok // super_tok       # 1024-token iterations (16)
    tb = super_tok // P          # tokens per output partition per iter (8)
    dc = D // P                  # contraction chunks (2 x 128)
    gr = 2 * super_tok           # input DMA granule: 2048 tokens, 4KB runs
    ngr = tok // gr

    nc = bacc.Bacc(
        "TRN2", target_bir_lowering=False, debug=False, num_devices=num_devices
    )
    xt_d = nc.dram_tensor("xt", [D, tok], BF16, kind="ExternalInput").ap()
    ct_d = nc.dram_tensor("ct", [D, D], BF16, kind="ExternalInput").ap()
    out_d = nc.dram_tensor("out", [tok, D], BF16, kind="ExternalOutput").ap()

    with ExitStack() as ctx:
        tc = ctx.enter_context(tile.TileContext(nc))
        consts = ctx.enter_context(tc.tile_pool(name="consts", bufs=1))
        xin_pool = ctx.enter_context(tc.tile_pool(name="xin", bufs=6))
        out_sb_pool = ctx.enter_context(tc.tile_pool(name="out_sb", bufs=4))
        # Each PSUM tile spans 2 banks ([128, 1024] fp32); 4 bufs = all
        # 8 banks, 2 iterations in flight.
        out_ps_pool = ctx.enter_context(
            tc.tile_pool(name="out_ps", bufs=4, space="PSUM")
        )

        # CT as [p, c, k] (d = c*128 + p), first on the scalar ring — the
        # very first matmul needs it. Two DMAs so the c=0 chunk lands in
        # half the time (matmuls are ordered c0-first).
        ct_sb = consts.tile([P, dc, D], BF16)
        ct_r = ct_d.rearrange("(c p) k -> p c k", p=P)
        nc.scalar.dma_start(ct_sb[:, 0:1, :], ct_r[:, 0:1, :])
        nc.scalar.dma_start(ct_sb[:, 1:2, :], ct_r[:, 1:2, :])

        # xT column j of iteration I holds token I*1024 + p*8 + s where
        # j = s*128 + p (host-side permutation) -> psum block s lands
        # tokens contiguously per partition for 4KB-run out DMAs.
        x_half = xt_d.rearrange("(c p) (h t) -> h p c t", p=P, t=super_tok // 2)
        x_fill = xt_d.rearrange("(c p) (i t) -> i p c t", p=P, t=super_tok)
        x_gr = xt_d.rearrange("(c p) (g t) -> g p c t", p=P, t=gr)
        o_t = out_d.rearrange("(i p s) k -> i p s k", p=P, s=tb)

        x_q = xt_d.rearrange("(c p) (q t) -> q p c t", p=P, t=super_tok // 4)

        xins = {}

        def stage_a_fill0():
            """Iteration 0 lands as 256/256/512-token tiles so the first
            matmuls start after only 128KB of input."""
            qa = xin_pool.tile([P, dc, super_tok // 4], BF16, name="xf0a")
            qb = xin_pool.tile([P, dc, super_tok // 4], BF16, name="xf0b")
            hc = xin_pool.tile([P, dc, super_tok // 2], BF16, name="xf0c")
            nc.sync.dma_start(qa[:], x_q[0])
            nc.sync.dma_start(qb[:], x_q[1])
            nc.sync.dma_start(hc[:], x_half[1])
            xins[0] = ("quads", qa, qb, hc)

        def stage_a_fill(i, eng):
            xc = xin_pool.tile([P, dc, super_tok], BF16, name=f"xfill{i}")
            eng.dma_start(xc[:], x_fill[i])
            xins[i] = ("one", xc, 0)

        # Granule ring schedule: sync takes g2 and g5 (its fill work ends
        # early), gpsimd the rest; sync then absorbs late out DMAs.
        GR_SYNC = {2, 5}

        def stage_a(g):
            """Granule g covers iterations 2g, 2g+1 (g >= 1)."""
            if not (1 <= g < ngr):
                return
            xg = xin_pool.tile([P, dc, gr], BF16)
            eng = nc.sync if g in GR_SYNC else nc.gpsimd
            eng.dma_start(xg[:], x_gr[g])
            xins[2 * g] = ("one", xg, 0)
            xins[2 * g + 1] = ("one", xg, super_tok)

        pss_by_iter = {}

        def stage_b(i):
            """16 matmuls -> 2 two-bank PSUM tiles (4 accumulation groups
            each), ordered c0-first so iteration 0 only gates on the c=0
            halves of ct and the fill."""
            if not (0 <= i < nit):
                return
            ent = xins.pop(i)
            if ent[0] == "quads":
                _, qa, qb, hc = ent
                q = tb // 4

                def wslice(c, s):
                    if s < q:
                        return qa[:, c, s * P:(s + 1) * P]
                    if s < 2 * q:
                        return qb[:, c, (s - q) * P:(s - q + 1) * P]
                    o = (s - 2 * q) * P
                    return hc[:, c, o:o + P]
            else:
                _, xg, off = ent

                def wslice(c, s):
                    o = off + s * P
                    return xg[:, c, o:o + P]

            pss = []
            for sp in range(2):
                ps = out_ps_pool.tile([P, (tb // 2) * D], F32)
                pss.append(ps)
                for s_in in range(tb // 2):
                    s = (tb // 2) * sp + s_in
                    for c in range(dc):
                        nc.tensor.matmul(
                            ps[:, s_in * D:(s_in + 1) * D],
                            wslice(c, s),
                            ct_sb[:, c, :],
                            start=(c == 0),
                            stop=(c == dc - 1),
                        )
            pss_by_iter[i] = pss

        # Out-DMA ring per iteration: scalar by default; sync takes the
        # late iterations once its input granules are through.
        OUT_SYNC = {9, 11, 13}

        def stage_c(i):
            """PSUM->SBUF bf16 copies (1 DVE + 1 ACT) + out DMA."""
            if not (0 <= i < nit):
                return
            pss = pss_by_iter.pop(i)
            out_sb = out_sb_pool.tile([P, tb, D], BF16)
            half = tb // 2
            if i >= nit - 2:
                # Tail drain: ship each half as soon as its copy lands,
                # on different rings.
                ringa = nc.scalar if i % 2 == 0 else nc.sync
                ringb = nc.sync if i % 2 == 0 else nc.scalar
                nc.vector.tensor_copy(out_sb[:, 0:half, :], pss[0][:])
                ringa.dma_start(o_t[i, :, 0:half, :], out_sb[:, 0:half, :])
                nc.scalar.copy(out_sb[:, half:tb, :], pss[1][:])
                ringb.dma_start(o_t[i, :, half:tb, :], out_sb[:, half:tb, :])
            else:
                ring = nc.sync if i in OUT_SYNC else nc.scalar
                nc.vector.tensor_copy(out_sb[:, 0:half, :], pss[0][:])
                nc.scalar.copy(out_sb[:, half:tb, :], pss[1][:])
                ring.dma_start(o_t[i], out_sb[:])

        stage_a_fill0()
        stage_a_fill(1, nc.gpsimd)
        stage_a(1)
        for i in range(nit + 1):
            if i % 2 == 0:
                stage_a(i // 2 + 2)
            stage_b(i)
            stage_c(i - 1)

    nc.compile()
    return nc


_PROGRAM_CACHE: dict = {}


def _get_program() -> bass.Bass:
    if "nc" not in _PROGRAM_CACHE:
        _PROGRAM_CACHE["nc"] = build_program()
    return _PROGRAM_CACHE["nc"]


def make_in_maps(x_flat: np.ndarray) -> list[dict]:
    """x_flat: [B*N, D] float32. Cast to bf16 and pre-transpose each shard
    to xT [d, j] where column j = I*1024 + s*128 + p holds token
    I*1024 + p*8 + s (matches the device's psum-block tiling)."""
    ct = np.ascontiguousarray(dct_matrix().T).astype(NP_BF16)  # [d, k]
    xb = x_flat.astype(NP_BF16)
    nit = TOK_PER_CORE // 1024
    # [core, I, p, s, d] -> [core, d, I, s, p]
    xr = xb.reshape(N_CORES, nit, P, 8, D).transpose(0, 4, 1, 3, 2)
    xt = np.ascontiguousarray(xr).reshape(N_CORES, D, TOK_PER_CORE)
    return [{"xt": xt[i], "ct": ct} for i in range(N_CORES)]


def kernel(x: np.ndarray) -> np.ndarray:
    x = np.ascontiguousarray(np.asarray(x, dtype=np.float32))
    b, n, d = x.shape
    assert (b, n, d) == (B, N, D), f"unexpected shape {x.shape}"
    nc = _get_program()
    in_maps = make_in_maps(x.reshape(b * n, d))
    res = run_bass_kernel_spmd(nc, in_maps, core_ids=list(range(N_CORES)))
    out = np.concatenate(
        [np.asarray(r["out"]).astype(np.float32) for r in res.results], axis=0
    )
    return out.reshape(b, n, d)
